# revision 1
# baseline (speedup 1.0000x reference)
"""Dynamic depthwise-3x3 conv (AClayer) on 8 TRN2 NeuronCores.

Structure: out[n,ch,i,j] = sum_p w[n,ch,p] * xpad[n,ch,i+di(p),j+dj(p)]
where w[n,ch,:] = BN(conv1x1(avgpool16x16(x)))[n,:,ch//16,ch%16].

Sharding: core k = (sample k//2, row-half k%2), all 256 channels.
Two NEFF launches:
  Phase A: each core pools its shard via PE matmuls (conv-before-pool swap:
           sigma_raw[o,s] = sum_c w_conv[o,c] * sum_{8x8} x[c,...]) ->
           sigma_loc [9,128] f32 out.
  Host:    relays the 8 tiny sigma blocks to every core (rotated so each
           core's own sample sits first). No math on host.
  Phase B: each core redundantly computes global BN stats (exact sync-BN;
           eps folded for the un-divided pooling sums), builds per-channel
           9-tap weights, and runs the stencil: channels on partitions,
           PE does rows [0,R_PE) via diagonal-matmul accumulation in PSUM
           (ACT evacuates), DVE does rows [R_PE,64) via tensor_scalar(4x) +
           tensor_tensor(2x) chains in bf16.

All DRAM inputs are host-packed chunk-major so every DMA reads a fully
contiguous HBM range (strided reads run at ~half bandwidth).
"""
from contextlib import ExitStack

import numpy as np
import ml_dtypes

import concourse.bass as bass
import concourse.mybir as mybir
from concourse.bass_utils import run_bass_kernel_spmd

bf16 = ml_dtypes.bfloat16
DT = mybir.dt
Alu = mybir.AluOpType
ActF = mybir.ActivationFunctionType

N_CORES = 8
CORE_IDS = list(range(N_CORES))

C, H, W = 256, 128, 128
RS, CS = 66, 130              # shard rows (with halo), padded cols
SH = RS * CS                  # 8580 elems per channel per shard
R_PE = 36                     # PE-region output rows per channel-block
R_DVE = 64 - R_PE
AROWS = R_PE + 2              # phase-B chunk A rows (covers PE region taps)
ALEN = AROWS * CS             # 5460
BLEN = SH - ALEN              # 3120
XS_LEN = (R_DVE + 2) * CS     # shifted-copy span (26 rows)
EPS_EFF = 4096 * 1e-5         # eps for un-divided (x64) pooling sums
# PE chunk -> psum bank (rotated between blocks so a group never waits on
# the evacuation of the group right before it); cumulative s_act waits
BANK_CB = {0: [0, 1, 2, 3, 4, 5, 6, 7, 0],
           1: [2, 3, 4, 5, 6, 7, 0, 1, 2]}
GROUPS = [(0, 1, 2, 3), (4, 5, 6, 7), (8,)]
ACT_WAIT = [0, 0, 1, 2, 3, 4]   # s_act threshold before group G may run
# tap order for the DVE region: the 6 even-offset taps direct from X,
# the 3 dj=1 taps (odd offset) via the 1-element-shifted copy XS
DVE_TAPS = [(0, 0), (0, 2), (1, 0), (1, 2), (2, 0), (2, 2)]
XS_TAPS = [(0, 1), (1, 1), (2, 1)]


def build_phase_a():
    nc = bass.Bass()
    # x: 4 contiguous chunks (cb0h0, cb1h0, cb0h1, cb1h1), interior rows
    # 1..64 cols 1..128 only -> [128, 32*128] each
    x = nc.declare_dram_parameter("x", [4, 128, 4096], DT.bfloat16,
                                  isOutput=False)
    wt = nc.declare_dram_parameter("wt", [128, 18], DT.bfloat16,
                                   isOutput=False)
    sig = nc.declare_dram_parameter("sig", [9, 128], DT.float32, isOutput=True)

    with (
        nc.sbuf_tensor("XA", [128, 4 * 4096], DT.bfloat16) as XA,
        nc.sbuf_tensor("WT", [128, 18], DT.bfloat16) as WT,
        nc.sbuf_tensor("SIG", [9, 128], DT.float32) as SIG,
        nc.psum_tensor("PS", [9, 1024], DT.float32) as PS,
        nc.psum_tensor("PSW", [9, 512], DT.float32) as PSW,
        nc.semaphore("s_in") as s_in,
        nc.semaphore("s_mm") as s_mm,
        nc.semaphore("s_red") as s_red,
        nc.semaphore("s_out") as s_out,
        nc.Block() as block,
    ):
        @block.sync
        def _(sync):
            sync.dma_start(out=WT[:, :], in_=wt[:, :]).then_inc(s_in, 16)
            for i in range(4):
                sync.dma_start(out=XA[:, i * 4096:(i + 1) * 4096],
                               in_=x[i]).then_inc(s_in, 16)
            sync.wait_ge(s_red, 2)
            sync.dma_start(out=sig[:, :], in_=SIG[:, :]).then_inc(s_out, 16)
            sync.wait_ge(s_out, 16)

        @block.tensor
        def _(te):
            te.wait_ge(s_in, 16)
            for _ in range(10):  # HAM warm-up on garbage data
                te.matmul(PSW[:, 0:512], lhsT=WT[:, 0:9], rhs=XA[:, 0:512],
                          start=True, stop=True)
            for half in (0, 1):
                te.wait_ge(s_in, 48 if half == 0 else 80)
                for cb in (0, 1):
                    lhsT = WT[:, cb * 9: cb * 9 + 9]
                    coff = (2 * half + cb) * 4096
                    for pr in range(4 * half, 4 * half + 4):
                        for dr in range(8):
                            r = (8 * pr + dr) % 32
                            rhs = XA[:, coff + r * 128: coff + r * 128 + 128]
                            # start=True clears the WHOLE psum bank, so only
                            # the first matmul touching each bank may set it
                            mm = te.matmul(
                                PS[:, pr * 128:(pr + 1) * 128], lhsT=lhsT,
                                rhs=rhs,
                                start=(pr % 4 == 0 and cb == 0 and dr == 0),
                                stop=(pr % 4 == 3 and cb == 1 and dr == 7),
                                skip_group_check=True)
            mm.then_inc(s_mm, 1)

        @block.vector
        def _(ve):
            ve.wait_ge(s_mm, 1)
            src = PS[:, :].rearrange("o (q dc) -> o q dc", dc=8)
            # self-sem hop so SIG's write tail is committed before the
            # out-DMA (sem'd via s_red) reads it
            ve.tensor_reduce(SIG[:, :], src, axis=mybir.AxisListType.X,
                             op=Alu.add).then_inc(s_red, 1)
            ve.wait_ge(s_red, 1)
            ve.nop().then_inc(s_red, 1)

    return nc


def build_phase_b():
    nc = bass.Bass()
    xa = nc.declare_dram_parameter("xa", [2, 128, ALEN], DT.bfloat16,
                                   isOutput=False)
    xb = nc.declare_dram_parameter("xb", [2, 128, BLEN], DT.bfloat16,
                                   isOutput=False)
    xs = nc.declare_dram_parameter("xs", [2, 128, XS_LEN - 1], DT.bfloat16,
                                   isOutput=False)
    sgb = nc.declare_dram_parameter("sgb", [9, 1026], DT.float32,
                                    isOutput=False)
    # contiguous per-region outputs (strided DRAM writes run at ~half BW)
    o_pe0 = nc.declare_dram_parameter("o_pe0", [128, R_PE * W], DT.bfloat16,
                                      isOutput=True)
    o_dv0 = nc.declare_dram_parameter("o_dv0", [128, R_DVE * W], DT.bfloat16,
                                      isOutput=True)
    o_pe1 = nc.declare_dram_parameter("o_pe1", [128, R_PE * W], DT.bfloat16,
                                      isOutput=True)
    o_dv1 = nc.declare_dram_parameter("o_dv1", [128, R_DVE * W], DT.bfloat16,
                                      isOutput=True)

    with ExitStack() as ctx:
        e = ctx.enter_context
        X = e(nc.sbuf_tensor("X", [128, 2 * SH], DT.bfloat16))
        XS = e(nc.sbuf_tensor("XS", [128, 2 * XS_LEN], DT.bfloat16))
        OA = e(nc.sbuf_tensor("OA", [128, 2 * 8192], DT.bfloat16))
        ACC = e(nc.sbuf_tensor("ACC", [128, R_DVE * 128], DT.bfloat16))
        TMP = e(nc.sbuf_tensor("TMP", [128, R_DVE * 128], DT.bfloat16))
        TA = e(nc.sbuf_tensor("TA", [128, 6 * R_DVE * 128], DT.bfloat16))
        SGB = e(nc.sbuf_tensor("SGB", [9, 1026], DT.float32))
        SQ = e(nc.sbuf_tensor("SQ", [9, 1024], DT.float32))
        ST = e(nc.sbuf_tensor("ST", [9, 12], DT.float32))
        WSM = e(nc.sbuf_tensor("WSM", [9, 256], DT.float32))
        WCH = e(nc.sbuf_tensor("WCH", [128, 18], DT.float32))
        DIAG = e(nc.sbuf_tensor("DIAG", [128, 18 * 128], DT.bfloat16))
        EYE128 = e(nc.sbuf_tensor("EYE128", [128, 128], DT.bfloat16))
        EYE9 = e(nc.sbuf_tensor("EYE9", [9, 9], DT.float32))
        PT = e(nc.psum_tensor("PT", [128, 4096], DT.float32))
        s_in = e(nc.semaphore("s_in"))
        s_xs = e(nc.semaphore("s_xs"))
        s_eye = e(nc.semaphore("s_eye"))
        s_s12 = e(nc.semaphore("s_s12"))
        s_bn1 = e(nc.semaphore("s_bn1"))
        s_act1 = e(nc.semaphore("s_act1"))
        s_bn2 = e(nc.semaphore("s_bn2"))
        s_tp = e(nc.semaphore("s_tp"))
        s_wch = e(nc.semaphore("s_wch"))
        s_pe = e(nc.semaphore("s_pe"))
        s_act = e(nc.semaphore("s_act"))
        s_dve = e(nc.semaphore("s_dve"))
        s_out = e(nc.semaphore("s_out"))
        s_v = e(nc.semaphore("s_v"))
        s_a = e(nc.semaphore("s_a"))
        s_wc = e(nc.semaphore("s_wc"))
        s_ta = e(nc.semaphore("s_ta"))
        block = e(nc.Block())

        @block.sync
        def _(sync):
            sync.dma_start(out=SGB[:, :], in_=sgb[:, :]).then_inc(s_in, 16)
            for cb in (0, 1):
                sync.dma_start(out=X[:, cb * SH: cb * SH + ALEN],
                               in_=xa[cb]).then_inc(s_in, 16)
                sync.dma_start(out=X[:, cb * SH + ALEN:(cb + 1) * SH],
                               in_=xb[cb]).then_inc(s_in, 16)
                sync.dma_start(
                    out=XS[:, cb * XS_LEN: cb * XS_LEN + XS_LEN - 1],
                    in_=xs[cb]).then_inc(s_xs, 16)
            # output DMAs, gated on compute completion
            sync.wait_ge(s_act, 3)
            sync.dma_start(out=o_pe0[:, :],
                           in_=OA[:, 0:R_PE * 128]).then_inc(s_out, 16)
            sync.wait_ge(s_dve, 1)
            sync.dma_start(out=o_dv0[:, :],
                           in_=OA[:, R_PE * 128:8192]).then_inc(s_out, 16)
            sync.wait_ge(s_act, 6)
            sync.dma_start(out=o_pe1[:, :],
                           in_=OA[:, 8192:8192 + R_PE * 128]
                           ).then_inc(s_out, 16)
            sync.wait_ge(s_dve, 2)
            sync.dma_start(out=o_dv1[:, :],
                           in_=OA[:, 8192 + R_PE * 128:16384]
                           ).then_inc(s_out, 16)
            sync.wait_ge(s_out, 64)

        @block.gpsimd
        def _(gp):
            # memset -> affine_select is a same-engine RAW on tiny ops:
            # self-semaphore the hop (see BN-chain note in the vector block)
            gp.memset(EYE128[:, :], 0.0)
            gp.memset(EYE9[:, :], 0.0).then_inc(s_eye, 1)
            gp.wait_ge(s_eye, 1)
            gp.affine_select(out=EYE128[:, :], in_=EYE128[:, :],
                             compare_op=Alu.not_equal, fill=1.0, base=0,
                             pattern=[[-1, 128]], channel_multiplier=1)
            gp.affine_select(out=EYE9[:, :], in_=EYE9[:, :],
                             compare_op=Alu.not_equal, fill=1.0, base=0,
                             pattern=[[-1, 9]], channel_multiplier=1)
            gp.nop().then_inc(s_eye, 1)

        @block.scalar
        def _(sc):
            # dummy sqrt to trigger the ACT table load early
            sc.activation(ST[:, 7:8], ST[:, 6:7], ActF.Sqrt)
            sc.wait_ge(s_bn1, 1)
            # self-sem hop: let the sqrt's write commit before signaling
            sc.activation(ST[:, 7:8], ST[:, 6:7], ActF.Sqrt).then_inc(s_a, 1)
            sc.wait_ge(s_a, 1)
            sc.nop().then_inc(s_act1, 1)

            def evac(G, na):
                cb, gi = divmod(G, 3)
                sc.wait_ge(s_pe, G + 1)
                bank = BANK_CB[cb]
                for ch in GROUPS[gi]:
                    a = sc.activation(
                        OA[:, cb * 8192 + ch * 512:
                           cb * 8192 + ch * 512 + 512],
                        PT[:, bank[ch] * 512: bank[ch] * 512 + 512],
                        ActF.Copy)
                a.then_inc(s_a, 1)
                sc.wait_ge(s_a, na)
                sc.nop().then_inc(s_act, 1)

            def products(cb):
                # weighted products of the 3 odd-offset taps for the DVE
                # region, from the shifted copy (frees DVE's tensor_scalar)
                sc.wait_ge(s_in, 48 + 32 * cb)
                sc.wait_ge(s_xs, 16 + 16 * cb)
                xscb = XS[:, cb * XS_LEN:(cb + 1) * XS_LEN].rearrange(
                    "p (r c) -> p r c", c=CS)
                rd = R_DVE * 128
                for j, (di, dj) in enumerate(XS_TAPS):
                    tap = xscb[:, di: di + R_DVE, 0:128]
                    dst = TA[:, (3 * cb + j) * rd: (3 * cb + j + 1) * rd]
                    dst = dst.rearrange("p (r c) -> p r c", c=128)
                    sc.activation(dst, tap, ActF.Copy,
                                  scale=WCH[:, cb * 9 + 3 * di + dj:
                                            cb * 9 + 3 * di + dj + 1]
                                  ).then_inc(s_ta, 1)

            sc.wait_ge(s_wc, 1)
            products(0)
            evac(0, 2)
            products(1)
            for G in range(1, 6):
                evac(G, G + 2)

        @block.vector
        def _(ve):
            # back-to-back dependent DVE ops on tiny operands race (the next
            # op's reads overlap the previous op's in-flight writes), so the
            # whole BN small-op chain is self-semaphored hop by hop.
            vc = [0]

            def step(ins):
                vc[0] += 1
                ins.then_inc(s_v, 1)
                ve.wait_ge(s_v, vc[0])

            # BN statistics in the raw-sums basis, minimal serial depth:
            #   Dv = 1024*S2 - S1^2 = 1024^2 * var_raw
            #   s  = 1024*gamma / sqrt(Dv + 1024^2*eps)
            #   t  = beta - (S1/1024)*s
            ve.wait_ge(s_in, 16)
            ve.tensor_scalar(ST[:, 10:11], SGB[:, 1024:1025], 1024.0, None,
                             Alu.mult)                         # gamma*1024
            ve.tensor_reduce(ST[:, 0:1], SGB[:, 0:1024],
                             axis=mybir.AxisListType.X, op=Alu.add)   # S1
            ve.tensor_tensor(SQ[:, :], SGB[:, 0:1024], SGB[:, 0:1024],
                             Alu.mult)
            step(ve.tensor_reduce(ST[:, 1:2], SQ[:, :],
                                  axis=mybir.AxisListType.X, op=Alu.add))
            ve.tensor_tensor(ST[:, 2:3], ST[:, 0:1], ST[:, 0:1],
                             Alu.mult)                         # S1^2
            step(ve.tensor_scalar(ST[:, 3:4], ST[:, 1:2], 1024.0,
                                  1048576.0 * EPS_EFF, Alu.mult, Alu.add))
            step(ve.tensor_tensor(ST[:, 6:7], ST[:, 3:4], ST[:, 2:3],
                                  Alu.subtract))               # Dv + K
            ve.nop().then_inc(s_bn1, 1)
            ve.wait_ge(s_act1, 1)
            step(ve.reciprocal(ST[:, 8:9], ST[:, 7:8]))
            step(ve.tensor_tensor(ST[:, 9:10], ST[:, 8:9], ST[:, 10:11],
                                  Alu.mult))                   # s
            step(ve.tensor_tensor(ST[:, 4:5], ST[:, 0:1], ST[:, 9:10],
                                  Alu.mult))                   # u = S1*s
            step(ve.tensor_scalar(ST[:, 5:6], ST[:, 4:5], -1.0 / 1024, None,
                                  Alu.mult))
            step(ve.tensor_tensor(ST[:, 11:12], SGB[:, 1025:1026],
                                  ST[:, 5:6], Alu.add))        # t
            step(ve.tensor_scalar(WSM[:, :], SGB[:, 0:256], ST[:, 9:10],
                                  ST[:, 11:12], Alu.mult, Alu.add))
            ve.nop().then_inc(s_bn2, 1)
            ve.wait_ge(s_tp, 1)
            ve.tensor_copy(WCH[:, 0:9], PT[:, 0:9])
            step(ve.tensor_copy(WCH[:, 9:18], PT[:, 9:18]))
            ve.nop().then_inc(s_wc, 1)
            ve.wait_ge(s_eye, 2)
            for cb in (0, 1):
                for p in range(9):
                    i = cb * 9 + p
                    ve.tensor_scalar(DIAG[:, i * 128:(i + 1) * 128],
                                     EYE128[:, :], WCH[:, i:i + 1], None,
                                     Alu.mult).then_inc(s_wch, 1)
            # DVE stencil region: rows [R_PE, 64) of each channel block
            for cb in (0, 1):
                ve.wait_ge(s_in, 48 + 32 * cb)
                accv = ACC[:, :].rearrange("p (r c) -> p r c", c=128)
                tmpv = TMP[:, :].rearrange("p (r c) -> p r c", c=128)
                outv = OA[:, cb * 8192 + R_PE * 128: cb * 8192 + 8192]
                outv = outv.rearrange("p (r c) -> p r c", c=128)
                xcb = X[:, cb * SH:(cb + 1) * SH].rearrange(
                    "p (r c) -> p r c", c=CS)
                xscb = XS[:, cb * XS_LEN:(cb + 1) * XS_LEN].rearrange(
                    "p (r c) -> p r c", c=CS)
                rd = R_DVE * 128
                for i, (di, dj) in enumerate(DVE_TAPS):
                    tap = xcb[:, R_PE + di: R_PE + di + R_DVE, dj: dj + 128]
                    wsc = WCH[:, cb * 9 + 3 * di + dj:
                              cb * 9 + 3 * di + dj + 1]
                    if i == 0:
                        ve.tensor_scalar(accv, tap, wsc, None, Alu.mult)
                    else:
                        ve.tensor_scalar(tmpv, tap, wsc, None, Alu.mult)
                        ve.tensor_tensor(accv, tmpv, accv, Alu.add)
                for j in range(3):  # ACT-produced odd-offset tap products
                    ve.wait_ge(s_ta, 3 * cb + j + 1)
                    tav = TA[:, (3 * cb + j) * rd: (3 * cb + j + 1) * rd]
                    tav = tav.rearrange("p (r c) -> p r c", c=128)
                    dst = outv if j == 2 else accv
                    last = ve.tensor_tensor(dst, tav, accv, Alu.add)
                # self-sem hop so the OutA write tail is committed before
                # the out-DMA (sem'd via s_dve) reads it
                step(last)
                ve.nop().then_inc(s_dve, 1)

        @block.tensor
        def _(te):
            te.wait_ge(s_eye, 2)
            # HAM warm-up bursts: keep PE busy (never >3.4us idle) through
            # the BN-weights chain without blocking real work for long
            for _ in range(12):
                te.matmul(PT[:, 3584:4096], lhsT=EYE128[:, :],
                          rhs=X[:, 0:512], start=True, stop=True)
            te.wait_ge(s_bn2, 1)
            # both transposes land in psum bank 0: the second must not
            # re-clear the bank (start=True wipes the whole bank)
            te.matmul(PT[:, 0:9], lhsT=WSM[:, 0:128], rhs=EYE9[:, :],
                      is_transpose=True, start=True, stop=False,
                      skip_group_check=True)
            te.matmul(PT[:, 9:18], lhsT=WSM[:, 128:256], rhs=EYE9[:, :],
                      is_transpose=True, start=False, stop=True,
                      skip_group_check=True).then_inc(s_tp, 1)
            for _ in range(6):  # stay warm through the diag builds
                te.matmul(PT[:, 3584:4096], lhsT=EYE128[:, :],
                          rhs=X[:, 0:512], start=True, stop=True)
            for cb in (0, 1):
                te.wait_ge(s_wch, 9 + 9 * cb)
                te.wait_ge(s_in, 32 + 32 * cb)  # chunk A of this cb loaded
                xcb = X[:, cb * SH:(cb + 1) * SH].rearrange(
                    "p (r c) -> p r c", c=CS)
                for gi, grp in enumerate(GROUPS):
                    G = 3 * cb + gi
                    if ACT_WAIT[G]:
                        te.wait_ge(s_act, ACT_WAIT[G])
                    for p in range(9):
                        di, dj = p // 3, p % 3
                        lhsT = DIAG[:, (cb * 9 + p) * 128:
                                    (cb * 9 + p) * 128 + 128]
                        for ch in grp:
                            rhs = xcb[:, 4 * ch + di: 4 * ch + di + 4,
                                      dj: dj + 128]
                            mm = te.matmul(
                                PT[:, BANK_CB[cb][ch] * 512:
                                   BANK_CB[cb][ch] * 512 + 512],
                                lhsT=lhsT, rhs=rhs,
                                start=(p == 0), stop=(p == 8))
                    mm.then_inc(s_pe, 1)

    return nc


def host_prep(x, w_conv):
    """Shard + pack all phase inputs (layout only, no math)."""
    n = x.shape[0]
    xpad = np.zeros((n, C, H + 2, W + 2), np.float32)
    xpad[:, :, 1:-1, 1:-1] = x
    xbf = xpad.astype(bf16)
    wt = np.ascontiguousarray(
        w_conv.reshape(9, 2, 128).transpose(2, 1, 0).reshape(128, 18)
    ).astype(bf16)
    maps_a, maps_b = [], []
    for k in range(N_CORES):
        sh = xbf[k // 2, :, 64 * (k % 2):64 * (k % 2) + 66, :]  # (256,66,130)
        shv = np.ascontiguousarray(sh).reshape(2, 128, RS, CS)
        # phase A: interior rows 1..64, cols 1..128, chunk-major
        xi = shv[:, :, 1:65, 1:129]                 # (2,128,64,128)
        xa_in = np.ascontiguousarray(
            xi.reshape(2, 128, 2, 32 * 128).transpose(2, 0, 1, 3))
        maps_a.append({"x": xa_in.reshape(4, 128, 4096), "wt": wt})
        # phase B: per-cb contiguous chunks
        flat = shv.reshape(2, 128, SH)
        xa_b = np.ascontiguousarray(flat[:, :, 0:ALEN])
        xb_b = np.ascontiguousarray(flat[:, :, ALEN:SH])
        xs_b = np.ascontiguousarray(
            flat[:, :, R_PE * CS + 1: R_PE * CS + XS_LEN])
        maps_b.append({"xa": xa_b, "xb": xb_b, "xs": xs_b})
    return maps_a, maps_b


def sgb_for_cores(sig, gamma, beta):
    """sig: [8, 9, 128] raw per-core sigma -> per-core sgb arrays."""
    sig_all = sig.reshape(4, 2, 9, 128).transpose(0, 2, 1, 3).reshape(4, 9, 256)
    out = []
    for k in range(N_CORES):
        ni = k // 2
        order = [ni] + [j for j in range(4) if j != ni]
        sgb = np.zeros((9, 1026), np.float32)
        sgb[:, 0:1024] = sig_all[order].transpose(1, 0, 2).reshape(9, 1024)
        sgb[:, 1024] = gamma
        sgb[:, 1025] = beta
        out.append(sgb)
    return out


def assemble_output(res_b, n):
    outf = np.empty((n, C, H, W), np.float32)
    for k in range(N_CORES):
        r = res_b.results[k]
        ni, r0 = k // 2, 64 * (k % 2)
        for cb, (pe, dv) in enumerate((("o_pe0", "o_dv0"),
                                       ("o_pe1", "o_dv1"))):
            chs = slice(cb * 128, cb * 128 + 128)
            outf[ni, chs, r0:r0 + R_PE, :] = \
                np.asarray(r[pe]).reshape(128, R_PE, W).astype(np.float32)
            outf[ni, chs, r0 + R_PE:r0 + 64, :] = \
                np.asarray(r[dv]).reshape(128, R_DVE, W).astype(np.float32)
    return outf


_CACHE = {}


def kernel(x, w_conv, gamma, beta):
    x = np.asarray(x, dtype=np.float32)
    w_conv = np.asarray(w_conv, dtype=np.float32)
    gamma = np.asarray(gamma, dtype=np.float32)
    beta = np.asarray(beta, dtype=np.float32)

    if "A" not in _CACHE:
        _CACHE["A"] = build_phase_a()
        _CACHE["B"] = build_phase_b()

    maps_a, maps_b = host_prep(x, w_conv)
    res_a = run_bass_kernel_spmd(_CACHE["A"], maps_a, CORE_IDS)
    sig = np.stack([np.asarray(res_a.results[k]["sig"]) for k in CORE_IDS])
    sgbs = sgb_for_cores(sig, gamma, beta)
    for m, sgb in zip(maps_b, sgbs):
        m["sgb"] = sgb
    res_b = run_bass_kernel_spmd(_CACHE["B"], maps_b, CORE_IDS)
    return assemble_output(res_b, x.shape[0])

